# revision 1
# baseline (speedup 1.0000x reference)
"""BiLSTM-CRF loss kernel for 8x Trainium2 NeuronCores (Bass/Tile).

Contract: kernel(**inputs) takes the FULL unsharded inputs (numpy) and
returns the FULL scalar output, matching reference.reference().

Strategy (data-parallel over batch, 8 cores x 64 sentences):
  - emissions^T[k,(u,b)] = W^T @ hidden^T via fp8 PE matmuls; hidden is
    transposed on the host to (H, 2, 256, BL): stream s=0 holds t = u
    (forward), stream s=1 holds t = 511-u (backward, time-reversed). W is
    pre-scaled by 8 on the host (fp8 dynamic range); the exp activation
    compensates with scale=1/8.
  - CRF log-partition via a product-domain BIDIRECTIONAL scan meeting at
    t=255 (Z = sum_i alpha_255[i] * beta_255[i]):
        fwd:  p <- (E'^T p) (x) e_t         (alpha, from t=0 up)
        bwd:  b <- E' (b (x) e_{t+1})       (beta, from t=511 down)
    with E' = exp(transitions) * e^{-SHIFT} in bf16. Both directions are
    packed into ONE 112-partition chain: state rows 0:48 = fwd, rows
    64:112 = bwd (rows 48:64 are architectural padding kept at zero), so
    each round is ONE PE matmul (block-diag stationary) + ONE DVE
    multiply. Renormalization is off-chain: column sums are staged to the
    host and 1/colsum is folded into an emissions slice two steps ahead
    (fold ops run on the Pool engine).
  - gold-path emission gather sum_{t,b} em[t,b,tags[t,b]]: Act stages the
    raw emissions to SBUF, the DVE forms the masked product against a
    host-built bf16 one-hot (2x mode), a single closed ones-matmul on the
    PE reduces each tile's partitions, and an Act Copy+accum_out collapses
    the row to a scalar (slots spaced 16 bytes apart: the accumulator
    store is a replicated 16-byte write).
  - start/end/transition gathers of the gold path are pure (tags, params)
    functions -> host scalars; final reduction in float64 on host.
"""

import os
import sys

import numpy as np

if "/opt/trn_rl_repo" not in sys.path:
    sys.path.insert(0, "/opt/trn_rl_repo")

import ml_dtypes

T, B, H, K = 512, 512, 512, 48
NCORES = 8
BL = B // NCORES          # batch per core
U = T // 2                # u-positions; u pairs (t=u, t=511-u)
UC = 16                   # u-positions per chunk
NCH = U // UC             # 16 chunks
TW = 512                  # free elems per emissions psum tile (= 8 u-cols)
UT = TW // BL             # u-positions per tile (8)
NTIL = U // UT            # 32 tiles
PB = 64                   # partition base of the bwd block
PT = 112                  # total partitions of the combined layout
SHIFT = 4.4               # e^-SHIFT folded into E' to keep the scan drift ~0
WSCALE = 8.0              # host-side W multiplier (fp8 range); exp scale 1/8
RENORM = 32
NREN_F = 7                # fwd renorms at t = 32..224
NREN_B = 7                # bwd renorms at t' = 288..480
FOLD_AHEAD = 6            # rounds of slack for the renorm fold chain
FIN_SLOT = 14
CSLOTS = 16
ROUNDS = 255              # combined rounds; round r consumes u = r+1

_COMPILED = None
LAST_RESULT = None        # BassKernelResults of the most recent run (for test.py)


def _build(reps=1, no_scan=False, no_emissions=False, no_gather=False,
           gather_mode="red"):
    from contextlib import ExitStack

    import concourse.tile as tile
    from concourse import bacc, mybir

    fp32 = mybir.dt.float32
    bf16 = mybir.dt.bfloat16
    fp8 = mybir.dt.float8e4
    AF = mybir.ActivationFunctionType
    ALU = mybir.AluOpType

    nc = bacc.Bacc(
        "TRN2", target_bir_lowering=False, debug=False, enable_asserts=False
    )
    hid = nc.dram_tensor("hidt", [H, U, 2, BL], fp8, kind="ExternalInput").ap()
    w = nc.dram_tensor("w", [H, K], fp8, kind="ExternalInput").ap()
    epc = nc.dram_tensor("epc", [PT, PT], bf16, kind="ExternalInput").ap()
    esten = nc.dram_tensor("esten", [PT, 1], fp32, kind="ExternalInput").ap()
    bia = nc.dram_tensor("bias", [PT, 1], fp32, kind="ExternalInput").ap()
    oh = nc.dram_tensor("onehot", [K, U, 2, BL], bf16, kind="ExternalInput").ap()
    cvec = nc.dram_tensor(
        "cvec", [1, CSLOTS * BL + 4 * NTIL], fp32, kind="ExternalOutput"
    ).ap()

    with tile.TileContext(nc) as tc:
        with ExitStack() as ctx:
            const = ctx.enter_context(tc.tile_pool(name="const", bufs=1))
            hidp = ctx.enter_context(tc.tile_pool(name="hid", bufs=18))
            ohp = ctx.enter_context(tc.tile_pool(name="oh", bufs=5))
            expp = ctx.enter_context(tc.tile_pool(name="expem", bufs=6))
            emp = ctx.enter_context(tc.tile_pool(name="embf", bufs=3))
            sttp = ctx.enter_context(tc.tile_pool(name="stt", bufs=2))
            state = ctx.enter_context(tc.tile_pool(name="state", bufs=3))
            e2p = ctx.enter_context(tc.tile_pool(name="e2", bufs=2))
            small = ctx.enter_context(tc.tile_pool(name="small", bufs=2))
            accp = ctx.enter_context(tc.tile_pool(name="acc", bufs=1))
            pse = ctx.enter_context(tc.tile_pool(name="pse", bufs=2, space="PSUM"))
            psf = ctx.enter_context(tc.tile_pool(name="psf", bufs=2, space="PSUM"))
            psr = ctx.enter_context(tc.tile_pool(name="psr", bufs=2, space="PSUM"))
            psacc = ctx.enter_context(
                tc.tile_pool(name="psacc", bufs=1, space="PSUM")
            )
            gjp = ctx.enter_context(tc.tile_pool(name="gjunk", bufs=2))

            # --- resident constants ---
            # (w first: the SP DMA queue is FIFO and tile-0 matmuls need it;
            # epc/esten/bias DMAs are emitted after chunk 0's, below)
            w_sb = const.tile([128, 4 * K], fp8)
            for hh in range(4):
                nc.sync.dma_start(
                    w_sb[:, hh * K : (hh + 1) * K], w[hh * 128 : (hh + 1) * 128, :]
                )
            epc_sb = const.tile([PT, PT], bf16)
            esten_sb = const.tile([PT, 1], fp32)
            bia_sb = const.tile([PT, 1], fp32)

            def const_dmas():
                nc.sync.dma_start(bia_sb[:], bia[:])
                nc.sync.dma_start(epc_sb[:], epc[:])
                nc.sync.dma_start(esten_sb[:], esten[:])
            ones_f = const.tile([PT, 1], bf16)
            nc.gpsimd.memset(ones_f[:, :], 0.0)
            nc.vector.memset(ones_f[0:K, :], 1.0)
            ones_b = const.tile([PT, 1], bf16)
            nc.gpsimd.memset(ones_b[:, :], 0.0)
            nc.vector.memset(ones_b[PB : PB + K, :], 1.0)
            ones_c = const.tile([K, 1], fp32)
            nc.vector.memset(ones_c[:], 1.0)
            ones_all = const.tile([PT, 1], bf16)
            nc.vector.memset(ones_all[:], 1.0)
            ones_fb = const.tile([PT, 1], bf16)
            nc.gpsimd.memset(ones_fb[:], 0.0)
            nc.vector.memset(ones_fb[0:K, :], 1.0)
            nc.vector.memset(ones_fb[PB : PB + K, :], 1.0)
            ones_r = const.tile([1, K], fp32)
            nc.vector.memset(ones_r[:], 1.0)
            ones_rp = const.tile([1, PT], fp32)
            nc.vector.memset(ones_rp[:], 1.0)
            cstage = accp.tile([1, CSLOTS * BL + 4 * NTIL], fp32)
            nc.gpsimd.memset(cstage[:], 1.0)

            # chunk_tiles[c] = (hts, oht, [expem_tile, expem_tile])
            chunk_tiles = [None] * NCH
            gp_box = [None]
            CF = UC * 2 * BL  # free elems per chunk, both streams (2048)

            def emit_dmas(c):
                u0 = c * UC
                hts = []
                for hh in range(4):
                    ht = hidp.tile([128, UC, 2, BL], fp8, tag="hid", name="hid_t")
                    nc.sync.dma_start(
                        ht[:], hid[hh * 128 : (hh + 1) * 128, u0 : u0 + UC, :, :]
                    )
                    hts.append(ht)
                # fwd one-hot at rows 0:48, bwd at rows 64:112 so each
                # gather stt sees equal base partitions for its two SBUF
                # operands (BIR verifier requirement)
                oht = ohp.tile([PT, UC, BL], bf16, tag="oh", name="oh_t")
                nc.sync.dma_start(oht[0:K, :, :], oh[:, u0 : u0 + UC, 0, :])
                nc.sync.dma_start(
                    oht[PB : PB + K, :, :], oh[:, u0 : u0 + UC, 1, :]
                )
                ets = [
                    expp.tile([PT, TW], bf16, tag="expem", name="expem_t")
                    for _ in range(UC // UT)
                ]
                chunk_tiles[c] = (hts, oht, ets)

            def emission_ops(c):
                """Generator of thunks; each emits one instruction."""
                if no_emissions:
                    for et in chunk_tiles[c][2]:
                        def mk_ms(et):
                            def f():
                                nc.gpsimd.memset(et[:], 1.0)
                            return f
                        yield mk_ms(et)
                    return
                hts, oht, ets = chunk_tiles[c]
                ps_box = [None, None]
                for j in range(UC // UT):
                    et = ets[j]
                    g = c * (UC // UT) + j  # global tile index
                    usl = slice(j * UT, (j + 1) * UT)

                    def mk_pad(et):
                        def f():
                            # rows 48:64 must be finite: the scan hadamard
                            # reads the full 112-partition slice. Start
                            # partition must be one of 0/32/64/96, so zero
                            # 32:64 and let the exp overwrite 32:48.
                            nc.gpsimd.memset(et[32:PB, :], 0.0)
                        return f
                    yield mk_pad(et)

                    def mk_mm(hh, half):
                        def f():
                            if hh == 0 and half == 0:
                                ps_box[0] = pse.tile(
                                    [PT, TW], fp32, tag="pse", name="ps_em"
                                )
                            base = 0 if half == 0 else PB
                            nc.tensor.matmul(
                                ps_box[0][base : base + K, :],
                                w_sb[:, hh * K : (hh + 1) * K],
                                hts[hh][:, usl, half, :],
                                start=(hh == 0),
                                stop=(hh == 3),
                            )
                        return f
                    for half in range(2):
                        for hh in range(4):
                            yield mk_mm(hh, half)

                    def mk_act(half, et):
                        def f():
                            base = 0 if half == 0 else PB
                            nc.scalar.activation(
                                et[base : base + K, :],
                                ps_box[0][base : base + K, :],
                                AF.Exp,
                                bias=bia_sb[base : base + K, :],
                                scale=1.0 / WSCALE,
                            )
                        return f
                    yield mk_act(0, et)
                    yield mk_act(1, et)

                    embf_box = [None]

                    def mk_copy(half):
                        def f():
                            if no_gather:
                                return
                            base = 0 if half == 0 else PB
                            if half == 0:
                                embf_box[0] = emp.tile(
                                    [PT, TW], bf16, tag="embf", name="embf_t"
                                )
                            # stage raw (8x-scaled) emissions to SBUF on the
                            # Act engine; GPSIMD cannot read PSUM
                            nc.scalar.copy(
                                embf_box[0][base : base + K, :],
                                ps_box[0][base : base + K, :],
                            )
                        return f
                    yield mk_copy(0)
                    yield mk_copy(1)

                    def mk_prod(half):
                        def f():
                            if no_gather or gather_mode == "copy":
                                return
                            base = 0 if half == 0 else PB
                            if half == 0:
                                ps_box[1] = sttp.tile(
                                    [PT, TW], bf16, tag="stt", name="masked_t"
                                )
                                # keep padding rows finite for the 112-wide
                                # ones-reduce (the product then overwrites
                                # rows 32:48)
                                nc.gpsimd.memset(ps_box[1][32:PB, :], 0.0)
                            # masked = em8 * onehot. On the DVE: all-bf16
                            # SBUF operands hit the 2x mode (~326ns), and
                            # real-hardware GPSIMD is ~2.6x slower than the
                            # cost model thinks, which saturated Pool
                            nc.vector.tensor_tensor(
                                ps_box[1][base : base + K, :],
                                embf_box[0][base : base + K, :],
                                oht[base : base + K, usl, :],
                                ALU.mult,
                            )
                        return f
                    yield mk_prod(0)
                    yield mk_prod(1)

                    def mk_red(g):
                        def f():
                            if no_gather or gather_mode in ("copy", "prod"):
                                return
                            # single closed matmul: 112-wide ones (zero at
                            # rows 48:64) sums both halves of the masked
                            # tile at once -- multi-matmul accumulation
                            # groups into a [1, N] row crash the runtime.
                            # One Act Copy+accum collapses the row to a
                            # scalar staged next to the colsums. ONE
                            # persistent row tile: the WAR dependency
                            # serializes the next tile's matmul behind this
                            # tile's accumulator read (rotating buffers
                            # raced on hardware).
                            gp = gp_box[0]
                            nc.tensor.matmul(
                                gp[0:1, :], ones_fb[:],
                                ps_box[1][0:PT, :],
                                start=True, stop=True,
                            )
                            gj = gjp.tile([1, TW], fp32, tag="gj", name="gj")
                            # the Act accumulator store is a 16-byte
                            # replicated write (aligned down): space slots
                            # 4 floats apart so neighbours don't clobber
                            slot = CSLOTS * BL + 4 * g
                            nc.scalar.activation(
                                gj[:], gp[0:1, :], AF.Copy,
                                accum_out=cstage[:, slot : slot + 1],
                            )
                        return f
                    yield mk_red(g)

            def e_slice(u):
                """Combined (112, 64) slice: rows 0:48 = e_u ; 64:112 =
                e_{511-u}; 48:64 = zeros."""
                et = chunk_tiles[u // UC][2][(u % UC) // UT]
                ul = u % UT
                return et[:, ul * BL : (ul + 1) * BL]

            prescaled = {}

            def e_used(u):
                return prescaled.pop(u) if u in prescaled else e_slice(u)

            def side_renorm(st, slot, target_u, half):
                """Stage colsum(one half of st) to cstage[slot]; fold
                1/colsum into that half of the combined emissions slice
                consumed at round using u=target_u. Off the scan chain.
                Returns a thunk (part B) to emit a few rounds later so the
                bc matmul never head-of-line-blocks the scan matmul in the
                in-order PE queue while it waits for the reciprocal."""
                base = 0 if half == 0 else PB
                onesx = ones_f if half == 0 else ones_b
                cs = psr.tile([PT, BL], fp32, tag="psr", name="cs_r")
                nc.tensor.matmul(
                    cs[:1, :], onesx[:], st[:], start=True, stop=True
                )
                nc.scalar.copy(cstage[:, slot * BL : (slot + 1) * BL], cs[:1, :])
                rec = small.tile([1, BL], fp32, tag="rec", name="rec")
                nc.vector.reciprocal(rec[:], cs[:1, :])

                def part_b():
                    bc = psr.tile([PT, BL], fp32, tag="psr", name="bc_r")
                    nc.tensor.matmul(
                        bc[base : base + K, :], ones_r[:], rec[:],
                        start=True, stop=True
                    )
                    src = e_slice(target_u)
                    e2 = e2p.tile([PT, BL], bf16, tag="e2", name="e_rn")
                    nc.gpsimd.tensor_copy(e2[:], src)
                    # bc is PSUM, which GPSIMD cannot read: multiply on DVE
                    nc.vector.tensor_mul(
                        e2[base : base + K, :],
                        bc[base : base + K, :],
                        src[base : base + K, :],
                    )
                    prescaled[target_u] = e2
                return part_b

            for rep in range(reps):
                gp_box[0] = psacc.tile([128, TW], fp32, tag="psacc",
                                       name="g_part")
                emit_dmas(0)
                if rep == 0:
                    const_dmas()
                pending = list(emission_ops(0))
                # eagerly run only what the scan init needs: tile 0's pad,
                # 8 matmuls and 2 exps (the first 11 thunks). Its gathers
                # and tile 1's ops interleave into the scan rounds.
                neager = 11 if not no_emissions else len(pending)
                for op in pending[:neager]:
                    op()
                pending = pending[neager:]

                if not no_scan:
                    # init: p_0 = e_0 (x) exp(start) ; q_511 = e_511 (x) exp(end)
                    e0 = e_slice(0)
                    st = state.tile([PT, BL], bf16, tag="p", name="p_init")
                    nc.vector.tensor_scalar_mul(st[:], e0, esten_sb[:])
                    if rep > 0:
                        # value-preserving dep on the previous rep's final
                        # output so multi-rep timing builds execute serially
                        bcf = psr.tile([PT, BL], fp32, tag="psr", name="bcf")
                        nc.tensor.matmul(
                            bcf[:], ones_rp[:],
                            cstage[:, FIN_SLOT * BL : (FIN_SLOT + 1) * BL],
                            start=True, stop=True,
                        )
                        st2i = state.tile([PT, BL], bf16, tag="p", name="p_ser")
                        nc.vector.scalar_tensor_tensor(
                            st2i[:], bcf[:], 0.0, st[:], ALU.mult, ALU.add
                        )
                        st = st2i

                    nfwd_r = 0
                    nbwd_r = 0
                    deferred = {}
                    for r in range(ROUNDS):
                        if r % UC == 0 and r // UC < NCH - 1:
                            emit_dmas(r // UC + 1)
                            pending += list(emission_ops(r // UC + 1))
                        per_step = (max(1, (len(pending) + UC - 1) // UC)
                                    if pending else 0)

                        t = r + 1          # fwd produces p_t ; bwd q_{511-t}
                        eu = e_used(t)     # (112, 64)
                        sf = psf.tile([PT, BL], fp32, tag="psf", name="ps_s")
                        nc.tensor.matmul(sf[:], epc_sb[:], st[:],
                                         start=True, stop=True)
                        st2 = state.tile([PT, BL], bf16, tag="p", name="p_s")
                        nc.vector.tensor_mul(st2[:], sf[:], eu)
                        st = st2

                        if t % RENORM == 0 and t // RENORM <= NREN_F:
                            deferred.setdefault(r + 3, []).append(
                                side_renorm(st, t // RENORM - 1,
                                            t + FOLD_AHEAD, 0))
                            nfwd_r += 1
                        tb = 511 - t
                        if tb % RENORM == 0 and 9 <= tb // RENORM <= 15:
                            deferred.setdefault(r + 3, []).append(
                                side_renorm(st, NREN_F + (15 - tb // RENORM),
                                            511 - tb + FOLD_AHEAD, 1))
                            nbwd_r += 1
                        for op in deferred.pop(r, []):
                            op()

                        for _ in range(per_step):
                            if pending:
                                pending.pop(0)()
                    while pending:
                        pending.pop(0)()
                    assert nfwd_r == NREN_F and nbwd_r == NREN_B, (nfwd_r, nbwd_r)
                    assert not prescaled, list(prescaled)

                    # meeting at t=255: Z = sum_i p_255[i] * b_255[i].
                    # Applying only the bwd-block COLUMNS of the combined
                    # stationary lands E'^T q_256 at partition base 0, so the
                    # multiply with p_255 (also base 0) is base-aligned.
                    sfx = psf.tile([PT, BL], fp32, tag="psf", name="ps_x")
                    nc.tensor.matmul(sfx[0:K, :], epc_sb[:, PB : PB + K],
                                     st[:], start=True, stop=True)
                    m = small.tile([K, BL], fp32, tag="meet", name="meet")
                    nc.vector.tensor_mul(m[:], sfx[0:K, :], st[0:K, :])
                    fin = psr.tile([PT, BL], fp32, tag="psr", name="fin")
                    nc.tensor.matmul(
                        fin[:1, :], ones_c[:], m[:], start=True, stop=True
                    )
                    nc.scalar.copy(
                        cstage[:, FIN_SLOT * BL : (FIN_SLOT + 1) * BL], fin[:1, :]
                    )
                else:
                    for c in range(1, NCH):
                        emit_dmas(c)
                        for op in emission_ops(c):
                            op()

                # inside the rep loop: on the HWDGE FIFO this gates the next
                # rep's input DMAs, serializing reps for latency measurement
                nc.sync.dma_start(cvec[:], cstage[:])

    nc.compile()
    return nc


def _get_compiled():
    global _COMPILED
    if _COMPILED is None:
        _COMPILED = _build()
    return _COMPILED


def _numpy_reference(hidden, W, b, start_transitions, end_transitions, transitions,
                     tags, mask):
    """Plain numpy fallback (only used if mask is not all ones)."""
    em = hidden.astype(np.float64) @ W.astype(np.float64) + b.astype(np.float64)
    maskf = mask.astype(np.float64)
    bar = np.arange(em.shape[1])
    st = start_transitions.astype(np.float64)
    en = end_transitions.astype(np.float64)
    tr = transitions.astype(np.float64)
    num = st[tags[0]] + em[0, bar, tags[0]]
    trs = tr[tags[:-1], tags[1:]]
    ems = np.take_along_axis(em[1:], tags[1:][..., None], axis=2)[..., 0]
    num = num + ((trs + ems) * maskf[1:]).sum(axis=0)
    seq_ends = mask.astype(np.int64).sum(axis=0) - 1
    num = num + en[tags[seq_ends, bar]]
    score = st[None, :] + em[0]
    for t in range(1, em.shape[0]):
        nxt = score[:, :, None] + tr[None] + em[t][:, None, :]
        m = nxt.max(axis=1)
        nxt = m + np.log(np.exp(nxt - m[:, None, :]).sum(axis=1))
        score = np.where(mask[t][:, None], nxt, score)
    fm = score + en[None, :]
    mm = fm.max(axis=1)
    denom = mm + np.log(np.exp(fm - mm[:, None]).sum(axis=1))
    return np.float32((num - denom).sum())


def kernel(hidden, W, b, start_transitions, end_transitions, transitions, tags,
           mask):
    hidden = np.asarray(hidden)
    W = np.asarray(W, dtype=np.float32)
    b = np.asarray(b, dtype=np.float32)
    start_transitions = np.asarray(start_transitions, dtype=np.float32)
    end_transitions = np.asarray(end_transitions, dtype=np.float32)
    transitions = np.asarray(transitions, dtype=np.float32)
    tags = np.asarray(tags)
    mask = np.asarray(mask)

    if not mask.all():
        return _numpy_reference(hidden, W, b, start_transitions, end_transitions,
                                transitions, tags, mask)

    from concourse.bass_utils import run_bass_kernel_spmd

    nc = _get_compiled()
    in_maps = _prepare_in_maps(hidden, W, b, start_transitions, end_transitions,
                               transitions, tags)

    global LAST_RESULT
    res = run_bass_kernel_spmd(nc, in_maps, core_ids=list(range(NCORES)))
    LAST_RESULT = res

    return _host_reduce(b, start_transitions, end_transitions, transitions, tags,
                        res.results)


def _fold_streams(arr_t_last):
    """(X, T, BL) -> (X, U, 2, BL): stream 0 = t ascending 0..255, stream 1 =
    t descending 511..256, interleaved per u."""
    x, t, bl = arr_t_last.shape
    out = np.empty((x, U, 2, bl), dtype=arr_t_last.dtype)
    out[:, :, 0, :] = arr_t_last[:, :U, :]
    out[:, :, 1, :] = arr_t_last[:, : U - 1 : -1, :]
    return out


def _prepare_in_maps(hidden, W, b, start_transitions, end_transitions,
                     transitions, tags):
    f8 = ml_dtypes.float8_e4m3
    w_f8 = (W * WSCALE).astype(f8)
    eprime64 = np.exp(transitions.astype(np.float64)) * np.exp(-SHIFT)
    epc = np.zeros((PT, PT), dtype=ml_dtypes.bfloat16)
    epc[0:K, 0:K] = eprime64.astype(ml_dtypes.bfloat16)
    epc[PB : PB + K, PB : PB + K] = (
        np.ascontiguousarray(eprime64.T).astype(ml_dtypes.bfloat16)
    )
    esten = np.zeros((PT, 1), dtype=np.float32)
    esten[0:K, 0] = np.exp(start_transitions)
    esten[PB : PB + K, 0] = np.exp(end_transitions)
    bias = np.zeros((PT, 1), dtype=np.float32)
    bias[0:K, 0] = b
    bias[PB : PB + K, 0] = b
    onehot = (
        (tags[None, :, :] == np.arange(K, dtype=tags.dtype)[:, None, None])
        .astype(ml_dtypes.bfloat16)
    )  # (K, T, B)

    in_maps = []
    for c in range(NCORES):
        sl = slice(c * BL, (c + 1) * BL)
        hidt = hidden[:, sl, :].transpose(2, 0, 1).astype(f8)
        in_maps.append(
            {
                "hidt": np.ascontiguousarray(_fold_streams(hidt)),
                "w": w_f8,
                "epc": epc,
                "esten": esten,
                "bias": bias,
                "onehot": np.ascontiguousarray(_fold_streams(onehot[:, :, sl])),
            }
        )
    return in_maps


def _host_reduce(b, start_transitions, end_transitions, transitions, tags,
                 results):
    tagsl = tags.astype(np.int64)
    total = np.float64(0.0)
    total += start_transitions.astype(np.float64)[tagsl[0]].sum()
    total += transitions.astype(np.float64)[tagsl[:-1], tagsl[1:]].sum()
    total += end_transitions.astype(np.float64)[tagsl[-1]].sum()
    total += b.astype(np.float64)[tagsl].sum()  # bias part of the em gather

    for c in range(NCORES):
        out = results[c]
        cva = out["cvec"].astype(np.float64)
        total += cva[0, CSLOTS * BL :: 4].sum() / WSCALE
        cv = cva[0, : CSLOTS * BL].reshape(CSLOTS, BL)
        denom_b = (
            np.log(cv[: NREN_F + NREN_B]).sum(axis=0)
            + np.log(cv[FIN_SLOT])
            + (T - 1) * SHIFT
        )
        total -= denom_b.sum()

    return np.float32(total)

